# revision 1
# baseline (speedup 1.0000x reference)
"""GAT (DGL GATConv) over complete per-doc graphs — Trainium2 Bass kernel, v2.

Problem: nn_CompletedSentenceGraph (gnn_message_passing).
  64 docs x 512 sentences, HIDDEN=256, HEADS=4, D=256.
  h = (x @ W).reshape(B,S,H,D)
  el/er = einsum(h, attn_l/attn_r)
  e[b,s,t,h] = leaky_relu(el[s]+er[t], 0.2); alpha = softmax over s
  out = einsum(alpha, h) + bias; return mean over heads  -> [N, 256]

Sharding: data-parallel over docs, 8 docs per core on 8 cores.

Math tricks (same family as v1):
  * exp(lrelu(x)) = max(exp(x), exp(0.2 x)); with x = el_s + er_t both exps
    are rank-1: expe'[s,t] = max(a_s, c_s * m_t), a=exp(el), c=exp(0.2 el),
    m=exp(-0.8 er) (per-dst scaling; softmax-invariant).
  * el/er from an augmented tiny matmul (WLR = W @ ALR built on device).
  * Z (softmax denominator) via near-free N=1 matmuls with a 4.0-ones rhs
    into a [128,4] psum -> one reciprocal per dst chunk; 4.0 folds the 1/H
    head-mean.
  * er-per-dst row obtained by PE-transposing the er columns of elr
    (no elrT matmul, no [4,512] psum bank).
  * Head combine: 3 psum reads on DVE + 1 scaled copy on ACT (gpsimd
    cannot access PSUM), final SBUF add on Pool; f32 accumulation, output
    DMA'd bf16 and converted to f32 on the host.
  * WLR / bias-mean precomputed on the host (numpy) and passed as extra
    inputs; per-doc x transposed via bf16 DRAM roundtrip + XBAR.
"""

from contextlib import ExitStack

import numpy as np

import concourse.mybir as mybir
import concourse.tile as tile
from concourse import bacc
from concourse.bass_utils import run_bass_kernel_spmd
from concourse.masks import make_identity

F32 = mybir.dt.float32
BF16 = mybir.dt.bfloat16
AX = mybir.AluOpType
ACTF = mybir.ActivationFunctionType

NUM_DOCS = 64
S = 512          # sentences per doc
K = 256          # hidden
H = 4            # heads
D = 256          # per-head out feats
N_CORES = 8
DPC = NUM_DOCS // N_CORES  # docs per core
P = 128

SS = S // P      # 4 s-subtiles per doc
KC = K // P      # 2 k-chunks
DC = S // P      # 4 dst chunks


def gat_tile_kernel(tc, xt, w, wlr, bias_m, out):
    """xt [DPC, 128, KC, 512] bf16 = host-pre-transposed x;
    w [256, 1024] bf16; wlr [256, 8] bf16 = host [W@attn_r | W@attn_l];
    bias_m [1, 256] bf16 = host mean-over-heads bias."""
    nc = tc.nc

    stack = ExitStack()
    with stack:
        consts = stack.enter_context(tc.tile_pool(name="consts", bufs=1))
        ps_small = stack.enter_context(
            tc.tile_pool(name="ps_small", bufs=1, space="PSUM"))
        # setup consts are emitted AFTER the first x loads (see below) so
        # the doc-0 x chain's DMAs get the head of the SP queue; setup_tmp
        # stays open for the whole program: closing it would emit a pool-exit
        # barrier that stalls the SP queue.
        setup_tmp = stack.enter_context(tc.tile_pool(name="setup_tmp", bufs=1))
        cst = {}

        def emit_setup():
            ident_f32 = consts.tile([P, P], F32)
            make_identity(nc, ident_f32)
            ident_bf = consts.tile([P, P], BF16)
            nc.gpsimd.tensor_copy(out=ident_bf, in_=ident_f32)

            # all weights arrive bf16 from the host: plain DMAs, no converts
            w_bf = consts.tile([P, KC, H * D], BF16)
            nc.sync.dma_start(out=w_bf, in_=w.rearrange("(kc p) f -> p kc f", p=P))
            wlr_bf = consts.tile([P, KC, 8], BF16)
            nc.sync.dma_start(out=wlr_bf,
                              in_=wlr.rearrange("(kc p) c -> p kc c", p=P))
            bias_mf = setup_tmp.tile([1, D], BF16)
            nc.sync.dma_start(out=bias_mf, in_=bias_m)
            bias_b = consts.tile([P, D], BF16)
            nc.gpsimd.partition_broadcast(bias_b, bias_mf)

            # ones column for the Z matmuls; 4.0 folds the 1/H head mean
            ones4 = consts.tile([P, 1], BF16)
            nc.gpsimd.memset(ones4, 4.0)
            cst.update(ident_f32=ident_f32, ident_bf=ident_bf, w_bf=w_bf,
                       wlr_bf=wlr_bf, bias_b=bias_b, ones4=ones4)

        # ---------------- per-doc pipeline ----------------
        with tc.tile_pool(name="xtp", bufs=4) as xtp, \
             tc.tile_pool(name="hp", bufs=3) as hp, \
             tc.tile_pool(name="ep", bufs=3) as ep, \
             tc.tile_pool(name="mp", bufs=3) as mp, \
             tc.tile_pool(name="sp", bufs=4) as sp, \
             tc.tile_pool(name="accp", bufs=4) as accp, \
             tc.tile_pool(name="ps_proj", bufs=3, space="PSUM") as ps_proj, \
             tc.tile_pool(name="ps_agg", bufs=3, space="PSUM") as ps_agg:

            def stage_x(d):
                """xT arrives pre-transposed/pre-bf16 from the host: 1 DMA."""
                xt_bf = xtp.tile([P, KC, S], BF16)
                nc.sync.dma_start(out=xt_bf, in_=xt[d])
                return xt_bf

            def stage_proj(d, xt_bf):
                """projection + h copies + el/er + m broadcast + expe.

                The el/er matmuls run FIRST so the m chain (transpose, exp,
                DRAM-broadcast) and the expe DVE ops overlap the pa/pb
                projection matmuls instead of serializing after them.
                """
                # el/er for all 4 s-subtiles up front: one [128, SS, 8] psum
                pcall = ps_small.tile([P, SS, 8], F32, tag="ptr")
                for ss in range(SS):
                    for kc in range(KC):
                        nc.tensor.matmul(pcall[:, ss, :],
                                         lhsT=xt_bf[:, kc, ss * P:(ss + 1) * P],
                                         rhs=cst['wlr_bf'][:, kc, :],
                                         start=(kc == 0), stop=(kc == KC - 1))
                # m-per-sentence: me16 = exp(-0.8*er) straight from psum,
                # written c-major (h outer, ss inner) so the bf16 PE
                # transpose sees one contiguous free dim
                me16 = sp.tile([P, 4 * SS], BF16, tag="me16")
                nc.scalar.activation(
                    out=me16,
                    in_=pcall[:, :, 0:4].rearrange("p s c -> p c s"),
                    func=ACTF.Exp, scale=-0.8)
                trm = ps_small.tile([4 * SS, P], BF16, tag="ptr")
                nc.tensor.transpose(trm, me16, cst['ident_bf'])
                m16sb = sp.tile([4 * SS, P], BF16, tag="m16sb")
                nc.vector.tensor_copy(out=m16sb, in_=trm)
                # collapse to one partition, then Pool broadcasts
                m_row = sp.tile([1, 4 * SS, P], BF16, tag="mrow")
                nc.sync.dma_start(out=m_row, in_=m16sb[:, None, :])
                m_all = mp.tile([P, 4 * SS, P], BF16, tag="mall")
                for h in range(H):
                    nc.gpsimd.partition_broadcast(
                        m_all[:, h * SS:(h + 1) * SS, :],
                        m_row[:, h * SS:(h + 1) * SS, :])

                # a = exp(el), c = exp(0.2 el)  [128, ss, 4] f32 scalars
                a_sc = sp.tile([P, SS, H], F32, tag="asc")
                c_sc = sp.tile([P, SS, H], F32, tag="csc")
                nc.scalar.activation(out=a_sc, in_=pcall[:, :, 4:8], func=ACTF.Exp)
                nc.scalar.activation(out=c_sc, in_=pcall[:, :, 4:8], func=ACTF.Exp,
                                     scale=0.2)

                # expe'[h] = max(a_s, c_s * m_t)   [128, ssub, dst] bf16
                expe = []
                for h in range(H):
                    eh = ep.tile([P, SS, S], BF16, tag=f"e{h}")
                    expe.append(eh)
                    for ss in range(SS):
                        nc.vector.tensor_scalar(
                            out=eh[:, ss, :],
                            in0=m_all[:, h * SS:(h + 1) * SS, :],
                            scalar1=c_sc[:, ss, h:h + 1],
                            scalar2=a_sc[:, ss, h:h + 1],
                            op0=AX.mult, op1=AX.max)

                # main projection + h copies
                ha = []
                for ss in range(SS):
                    h_t = hp.tile([P, H, D], BF16, tag=f"ha{ss}")
                    ha.append(h_t)
                    pa = ps_proj.tile([P, 512], F32, tag="pab")
                    pb = ps_proj.tile([P, 512], F32, tag="pab")
                    for kc in range(KC):
                        lt = xt_bf[:, kc, ss * P:(ss + 1) * P]
                        st = (kc == 0)
                        sp_ = (kc == KC - 1)
                        nc.tensor.matmul(pa, lhsT=lt, rhs=cst['w_bf'][:, kc, 0:512],
                                         start=st, stop=sp_)
                        nc.tensor.matmul(pb, lhsT=lt, rhs=cst['w_bf'][:, kc, 512:1024],
                                         start=st, stop=sp_)
                    nc.scalar.copy(out=h_t[:, 0:2, :],
                                   in_=pa.rearrange("p (h d) -> p h d", h=2))
                    nc.scalar.copy(out=h_t[:, 2:4, :],
                                   in_=pb.rearrange("p (h d) -> p h d", h=2))
                return ha, expe

            def stage_agg(d, ha, expe):
                """aggregation + Z + normalize + head-mean + out DMA."""
                acc = accp.tile([P, DC, D], BF16, tag="accb")
                # one Z psum for the whole doc: no per-dc psum reuse, so the
                # in-order PE queue never waits on the previous reciprocal
                pz = ps_small.tile([P, DC, H], F32, tag="pz")
                for dc in range(DC):
                    pu01 = ps_agg.tile([P, 2, D], F32, tag="pu")
                    pu23 = ps_agg.tile([P, 2, D], F32, tag="pu")
                    for h in range(H):
                        pu = (pu01 if h < 2 else pu23)[:, h % 2, :]
                        for sc in range(SS):
                            lt = expe[h][:, sc, dc * P:(dc + 1) * P]
                            nc.tensor.matmul(pu, lhsT=lt, rhs=ha[sc][:, h, :],
                                             start=(sc == 0), stop=(sc == SS - 1))
                            nc.tensor.matmul(pz[:, dc, h:h + 1], lhsT=lt,
                                             rhs=cst['ones4'],
                                             start=(sc == 0), stop=(sc == SS - 1))
                    rz = sp.tile([P, H], F32, tag="rz")
                    nc.vector.reciprocal(out=rz, in_=pz[:, dc, :])
                    accf = accp.tile([P, D], F32, tag="accf")
                    t2 = accp.tile([P, D], BF16, tag="t2")
                    t23 = accp.tile([P, D], F32, tag="t23")
                    # psum reads: 3 on DVE, 1 on ACT (gpsimd cannot access
                    # PSUM on hw); final SBUF-only add on Pool
                    nc.vector.scalar_tensor_tensor(
                        out=accf, in0=pu01[:, 0, :], scalar=rz[:, 0:1],
                        in1=cst['bias_b'], op0=AX.mult, op1=AX.add)
                    nc.vector.scalar_tensor_tensor(
                        out=accf, in0=pu01[:, 1, :], scalar=rz[:, 1:2],
                        in1=accf, op0=AX.mult, op1=AX.add)
                    nc.scalar.activation(
                        out=t2, in_=pu23[:, 0, :], func=ACTF.Copy,
                        scale=rz[:, 2:3])
                    nc.vector.scalar_tensor_tensor(
                        out=t23, in0=pu23[:, 1, :], scalar=rz[:, 3:4],
                        in1=t2, op0=AX.mult, op1=AX.add)
                    fineng = nc.vector
                    fineng.tensor_tensor(out=acc[:, dc, :], in0=accf,
                                         in1=t23, op=AX.add)
                for dc in range(DC):
                    nc.sync.dma_start(
                        out=out[d * S + dc * P:d * S + (dc + 1) * P, :],
                        in_=acc[:, dc, :])

            # software pipeline: x 2-3 ahead (3-doc prologue), proj 1 ahead
            # of agg
            xts = {}
            work = {}
            xts[0] = stage_x(0)
            emit_setup()
            for i in range(DPC):
                if i == 0:
                    for j in range(1, min(3, DPC)):
                        xts[j] = stage_x(j)
                elif i + 2 < DPC:
                    xts[i + 2] = stage_x(i + 2)
                work[i] = stage_proj(i, xts.pop(i))
                if i >= 1:
                    ha, expe = work.pop(i - 1)
                    stage_agg(i - 1, ha, expe)
            ha, expe = work.pop(DPC - 1)
            stage_agg(DPC - 1, ha, expe)


_NC_CACHE = None


def build_nc():
    global _NC_CACHE
    if _NC_CACHE is not None:
        return _NC_CACHE
    nc = bacc.Bacc("TRN2", target_bir_lowering=False, debug=False,
                   num_devices=N_CORES)
    xt = nc.dram_tensor("xt", [DPC, P, KC, S], BF16, kind="ExternalInput")
    w = nc.dram_tensor("w", [K, H * D], BF16, kind="ExternalInput")
    wlr = nc.dram_tensor("wlr", [K, 8], BF16, kind="ExternalInput")
    bias_m = nc.dram_tensor("bias_m", [1, D], BF16, kind="ExternalInput")
    out = nc.dram_tensor("out", [DPC * S, K], BF16, kind="ExternalOutput")
    with tile.TileContext(nc) as tc:
        gat_tile_kernel(tc, xt.ap(), w.ap(), wlr.ap(), bias_m.ap(), out.ap())
    nc.compile()
    _NC_CACHE = nc
    return nc


def kernel(sent_feature, W, attn_l, attn_r, bias, num_docs=NUM_DOCS, **_unused):
    sent_feature = np.asarray(sent_feature, dtype=np.float32)
    W = np.asarray(W, dtype=np.float32)
    attn_l = np.asarray(attn_l, dtype=np.float32)
    attn_r = np.asarray(attn_r, dtype=np.float32)
    bias = np.asarray(bias, dtype=np.float32)

    import ml_dtypes
    bf16 = ml_dtypes.bfloat16
    # host precompute: WLR[k, h] = sum_d W[k, h*D+d]*attn_r[h, d] (cols 0..3)
    # and attn_l (cols 4..7); bias mean over heads; x pre-transposed to
    # [doc, p, kc, s] bf16 (pure layout/dtype prep, per-core sharding).
    w4 = W.reshape(K, H, D)
    wlr = np.concatenate([
        np.einsum("khd,hd->kh", w4, attn_r),
        np.einsum("khd,hd->kh", w4, attn_l),
    ], axis=1).astype(bf16)
    bias_m = bias.reshape(H, D).mean(axis=0, keepdims=True).astype(bf16)
    w_bf = W.astype(bf16)
    xt_full = np.ascontiguousarray(
        sent_feature.reshape(NUM_DOCS, S, KC, P).transpose(0, 3, 2, 1)
    ).astype(bf16)

    nc = build_nc()
    in_maps = []
    for c in range(N_CORES):
        in_maps.append({
            "xt": xt_full[c * DPC:(c + 1) * DPC],
            "w": w_bf, "wlr": wlr, "bias_m": bias_m,
        })
    res = run_bass_kernel_spmd(nc, in_maps, core_ids=list(range(N_CORES)))
    out = np.concatenate([res.results[c]["out"] for c in range(N_CORES)], axis=0)
    return out.astype(np.float32)

